# revision 7
# baseline (speedup 1.0000x reference)
"""Multi-head attention (B=2, S=2048, D=1024, H=16) on 8 Trainium2 NeuronCores.

Sharding: core = b*4 + hg  (b = batch, hg = head-group of 4 heads).
Each core computes, for its batch b and its 4 heads:
    q^T = (Wq_g @ X_q^T + bq_g)      stored [256, 2048]  (head-dim on partitions)
    k^T likewise, v = X_v @ Wv_g^T + bv_g stored [2048, 4, 64]
    S^T[s_k, s_q] = k^T.T-contraction(d)  per head (row-packed PE pairs)
    P^T = exp(S^T / 8)                (scalar engine, exp only; no max-subtract)
    attnout^T[o, s_q] = v.T @ P^T     col-packed PE pairs (M=64 per head, two
                                      heads share one PE pass)
    rowsum[s_q]      = 1.T @ P^T      4-way col-packed M=1 matmuls into one
                                      PSUM bank (partitions 0/32/64/96)
    attnout_norm^T = attnout^T * (1/rowsum)  (DVE; reciprocal via DRAM-bounce
                                      transpose so it runs 128-lane-parallel)
    outT_partial[m, s] = Wo_g^T-contraction(o) @ attnout_norm^T   [1024, 2048]
Host gathers: out[b] = sum_g outT_partial.T + bo.

Engine split: scalar = exp only (it is the 1 elem/cycle/lane bottleneck);
DVE = all PSUM drains + normalize; gpsimd/sync = DMA queues.
All matmul inputs bf16 (PSUM accumulation f32).
"""

import numpy as np
import ml_dtypes

import concourse.bacc as bacc
import concourse.mybir as mybir
import concourse.tile as tile
from concourse.bass_utils import run_bass_kernel_spmd

BF16 = mybir.dt.bfloat16
F32 = mybir.dt.float32
AF = mybir.ActivationFunctionType
ALU = mybir.AluOpType

B, S, D = 2, 2048, 1024
H = 16
DK = 64
NCORES = 8
HG = 4  # head groups
HPG = 4  # heads per group
GO = HPG * DK  # 256 group output width

_NC = None


def _emit(nc, tc, io):
    xqT, xkT, xvT, wqT, wkT, wvT, woT, bqk, outT = (
        io["xqT"], io["xkT"], io["xvT"], io["wqT"], io["wkT"], io["wvT"],
        io["woT"], io["bqk"], io["outT"],
    )
    NIC = D // 128  # 8 contraction chunks of 128
    NSC = S // 128  # 16 s chunks of 128
    NSQ = S // 512  # 4 s chunks of 512

    with (
        tc.tile_pool(name="wp", bufs=1) as wp,
        tc.tile_pool(name="xp", bufs=1) as xp,
        tc.tile_pool(name="pp", bufs=1) as pp,
        tc.tile_pool(name="pt", bufs=8) as ptp,
        tc.tile_pool(name="nr", bufs=2) as nrp,
        tc.tile_pool(name="sc", bufs=2, space="PSUM") as psB,
        tc.tile_pool(name="dr", bufs=2, space="DRAM") as drp,
    ):
        # ---- load weights + inputs (in consumption order) ----
        bqk_t = wp.tile([128, 4], F32, name="bqk", tag="bqk")
        nc.sync.dma_start(bqk_t[:], bqk[:])
        ones_t = wp.tile([128, 1], BF16, name="ones", tag="ones")
        nc.vector.memset(ones_t[:], 1.0)
        # inputs alternate between the sync and gpsimd DMA queues: a single
        # queue sustains only ~220 GB/s, and the exp pipeline can't start
        # until xk+xq (8 MB) have landed
        wk = []
        xk = []
        for i in range(NIC):
            t = wp.tile([128, GO], BF16, name=f"wk{i}", tag=f"wk{i}")
            nc.sync.dma_start(t[:], wkT[128 * i:128 * i + 128, :])
            wk.append(t)
            t = xp.tile([128, S], BF16, name=f"xk{i}", tag=f"xk{i}")
            eng = nc.gpsimd if i % 2 == 0 else nc.sync
            eng.dma_start(t[:], xkT[128 * i:128 * i + 128, :])
            xk.append(t)
        wq = []
        xq = []
        for i in range(NIC):
            t = wp.tile([128, GO], BF16, name=f"wq{i}", tag=f"wq{i}")
            nc.sync.dma_start(t[:], wqT[128 * i:128 * i + 128, :])
            wq.append(t)
            t = xp.tile([128, S], BF16, name=f"xq{i}", tag=f"xq{i}")
            eng = nc.gpsimd if i % 2 == 0 else nc.sync
            eng.dma_start(t[:], xqT[128 * i:128 * i + 128, :])
            xq.append(t)
        wv = []
        xv = []
        for i in range(NIC):
            t = wp.tile([128, GO], BF16, name=f"wv{i}", tag=f"wv{i}")
            nc.sync.dma_start(t[:], wvT[128 * i:128 * i + 128, :])
            wv.append(t)
            t = xp.tile([128, S], BF16, name=f"xv{i}", tag=f"xv{i}")
            eng = nc.gpsimd if i % 2 == 0 else nc.sync
            eng.dma_start(t[:], xvT[128 * i:128 * i + 128, :])
            xv.append(t)
        wv_b = wp.tile([1, GO], BF16, name="wvb", tag="wvb")
        nc.sync.dma_start(wv_b[:], wvT[D:D + 1, :])
        xv_ones = xp.tile([1, S], BF16, name="xvo", tag="xvo")
        nc.sync.dma_start(xv_ones[:], xvT[D:D + 1, :])
        wo = []
        for oc in range(2):
            t = wp.tile([128, D], BF16, name=f"wo{oc}", tag=f"wo{oc}")
            nc.sync.dma_start(t[:], woT[128 * oc:128 * oc + 128, :])
            wo.append(t)

        kT = [pp.tile([128, S], BF16, name=f"kT{oc}", tag=f"kT{oc}") for oc in range(2)]
        qT = [pp.tile([128, S], BF16, name=f"qT{oc}", tag=f"qT{oc}") for oc in range(2)]
        v = [pp.tile([128, HPG, DK], BF16, name=f"v{sc}", tag=f"v{sc}") for sc in range(NSC)]

        # ---- projections in a scoped PSUM pool (4 banks, released before
        # the attention pools are allocated) ----
        with tc.tile_pool(name="pj", bufs=4, space="PSUM") as pj:
            # k^T: ic-outer with 4 accumulators per oc pass (accumulate as
            # the input DMAs land); bias added in the DVE drain.
            for oc in range(2):
                accs = [pj.tile([128, 512], F32, name="acc", tag="acc")
                        for _ in range(NSQ)]
                for ic in range(NIC):
                    for sc in range(NSQ):
                        nc.tensor.matmul(
                            accs[sc][:],
                            wk[ic][:, 128 * oc:128 * oc + 128],
                            xk[ic][:, 512 * sc:512 * sc + 512],
                            start=(ic == 0),
                            stop=(ic == NIC - 1),
                        )
                for sc in range(NSQ):
                    nc.vector.tensor_scalar(
                        kT[oc][:, 512 * sc:512 * sc + 512], accs[sc][:],
                        bqk_t[:, 2 + oc:3 + oc], None, op0=ALU.add,
                    )
            # q^T: sc-outer 2-acc mini-passes so the sqc=0 slice of qT (the
            # only one the first score blocks need) drains as early as
            # possible and the exp pipeline can start.
            for sc in range(NSQ):
                accs = [pj.tile([128, 512], F32, name="acc", tag="acc")
                        for _ in range(2)]
                for ic in range(NIC):
                    for oc in range(2):
                        nc.tensor.matmul(
                            accs[oc][:],
                            wq[ic][:, 128 * oc:128 * oc + 128],
                            xq[ic][:, 512 * sc:512 * sc + 512],
                            start=(ic == 0),
                            stop=(ic == NIC - 1),
                        )
                for oc in range(2):
                    nc.vector.tensor_scalar(
                        qT[oc][:, 512 * sc:512 * sc + 512], accs[oc][:],
                        bqk_t[:, oc:oc + 1], None, op0=ALU.add,
                    )
            # v: [128, 4, 64] per s-chunk (bias via ones-row matmul)
            for scg in range(NSC // NSQ):
                accs = [pj.tile([128, 512], F32, name="acc", tag="acc")
                        for _ in range(NSQ)]
                for ic in range(NIC):
                    for j in range(NSQ):
                        sc = scg * NSQ + j
                        nc.tensor.matmul(
                            accs[j][:, 0:GO],
                            xv[ic][:, 128 * sc:128 * sc + 128],
                            wv[ic][:],
                            start=(ic == 0),
                            stop=False,
                        )
                for j in range(NSQ):
                    sc = scg * NSQ + j
                    nc.tensor.matmul(
                        accs[j][:, 0:GO],
                        xv_ones[:, 128 * sc:128 * sc + 128],
                        wv_b[:],
                        start=False,
                        stop=True,
                    )
                    nc.vector.tensor_copy(
                        v[sc][:, :, :],
                        accs[j][:, 0:GO].rearrange("p (h d) -> p h d", d=DK),
                    )

        # ---- attention ----
        # pvacc pair tiles: partitions 0:64 = even head, 64:128 = odd head
        # (two col-packed matmuls share one PE pass).  rs tile: rowsums at
        # partitions 0/32/64/96 via 4-way col-packed M=1 matmuls.
        # pvacc gets its OWN 2 banks: PV(c+1) must depend only on the atc
        # drains of chunk c, never (via a shared slot ring) on the normalize
        # DMA chain or fproj — otherwise the pT queue fills and the exp
        # pipeline stalls ~20us at every chunk boundary.
        with (
            tc.tile_pool(name="pv", bufs=2, space="PSUM") as pvp,
            tc.tile_pool(name="fx", bufs=1, space="PSUM") as fxp,
            tc.tile_pool(name="rx", bufs=1, space="PSUM") as rxp,
        ):
            attnT = [pp.tile([128, S], BF16, name=f"at{oc}", tag=f"at{oc}")
                     for oc in range(2)]
            pvacc = {}
            rs = {}

            def normalize(c):
                atcs = []
                for p in range(2):
                    atc = nrp.tile([128, 512], F32, name="atc", tag="atc", bufs=4)
                    nc.vector.tensor_copy(atc[:], pvacc[c][p][:])
                    atcs.append(atc)
                rss = nrp.tile([128, 512], F32, name="rss", tag="rss", bufs=2)
                nc.vector.tensor_copy(rss[:], rs[c][:])
                # bounce rowsums through DRAM reshaped to [128, 16] so the
                # DVE reciprocal runs 128-lane-parallel
                rw_ = drp.tile([4, 512], F32, name="rw", tag="rw")
                for h in range(HPG):
                    nc.sync.dma_start(rw_[h:h + 1, :], rss[32 * h:32 * h + 1, :])
                rq_ = nrp.tile([128, 16], F32, name="rq", tag="rq", bufs=2)
                nc.sync.dma_start(
                    rq_.rearrange("p (h j) -> p h j", h=HPG),
                    rw_.rearrange("h (p j) -> p h j", p=128))
                rr_ = nrp.tile([128, 16], F32, name="rr", tag="rr", bufs=2)
                nc.vector.reciprocal(rr_[:], rq_[:])
                rd_ = drp.tile([4, 512], F32, name="rd", tag="rd")
                nc.sync.dma_start(
                    rd_.rearrange("h (p j) -> p h j", p=128),
                    rr_.rearrange("p (h j) -> p h j", h=HPG))
                for p in range(2):
                    rb_ = nrp.tile([128, 512], F32, name="rb", tag="rb", bufs=4)
                    nc.gpsimd.dma_start(
                        rb_[0:DK, :], rd_[2 * p:2 * p + 1, :].to_broadcast([DK, 512]))
                    nc.gpsimd.dma_start(
                        rb_[DK:128, :],
                        rd_[2 * p + 1:2 * p + 2, :].to_broadcast([DK, 512]))
                    st_ = nrp.tile([128, 512], BF16, name="st", tag="st", bufs=4)
                    nc.vector.tensor_mul(st_[:], atcs[p][:], rb_[:])
                    nc.gpsimd.dma_start(
                        attnT[p][:, 512 * c:512 * c + 512],
                        st_[:],
                    )

            def emit_fproj(c):
                for mc in range(D // 128):
                    fac = fxp.tile([128, 512], F32, name="fac", tag="fac")
                    for oc in range(2):
                        nc.tensor.matmul(
                            fac[:],
                            wo[oc][:, 128 * mc:128 * mc + 128],
                            attnT[oc][:, 512 * c:512 * c + 512],
                            start=(oc == 0),
                            stop=(oc == 1),
                        )
                    fo_ = nrp.tile([128, 512], BF16, name="fo", tag="fo", bufs=8)
                    nc.vector.tensor_copy(fo_[:], fac[:])
                    eng = nc.sync if mc % 2 == 0 else nc.gpsimd
                    eng.dma_start(
                        outT[128 * mc:128 * mc + 128, 512 * c:512 * c + 512],
                        fo_[:],
                    )

            def emit_pv(prev):
                pTs, c, k = prev
                if k == 0:
                    pvacc[c] = [
                        pvp.tile([128, 512], F32, name="pvacc", tag="pv")
                        for _ in range(2)
                    ]
                    rs[c] = rxp.tile([128, 512], F32, name="rs", tag="rs")
                for p in range(2):
                    for sub in range(2):
                        h = 2 * p + sub
                        nc.tensor.matmul(
                            pvacc[c][p][64 * sub:64 * sub + 64, :],
                            v[k][:, h, :],
                            pTs[h // 2][:, 512 * (h % 2):512 * (h % 2) + 512],
                            start=(k == 0),
                            stop=(k == NSC - 1),
                            tile_position=(0, 64 * sub),
                        )
                for h in range(HPG):
                    nc.tensor.matmul(
                        rs[c][32 * h:32 * h + 1, :],
                        ones_t[:, :],
                        pTs[h // 2][:, 512 * (h % 2):512 * (h % 2) + 512],
                        start=(k == 0),
                        stop=(k == NSC - 1),
                        tile_position=(0, 32 * h),
                    )
                if k == NSC - 1:
                    normalize(c)
                    emit_fproj(c)

            # 1-step software pipeline: scores/exp run one step ahead of the
            # PV matmuls; psB double-buffered per head-pair.
            prev = None
            for sqc in range(NSQ):
                for skc in range(NSC):
                    pTs = []
                    for hp in range(2):
                        ps_ = psB.tile([128, 1024], F32, name="sc", tag="sc")
                        for sub in range(2):
                            nc.tensor.matmul(
                                ps_[:, 512 * sub:512 * sub + 512],
                                kT[hp][64 * sub:64 * sub + 64,
                                       128 * skc:128 * skc + 128],
                                qT[hp][64 * sub:64 * sub + 64,
                                       512 * sqc:512 * sqc + 512],
                                start=True,
                                stop=True,
                                tile_position=(64 * sub, 0),
                            )
                        pT_ = ptp.tile([128, 1024], BF16, name="pT", tag="pT")
                        nc.scalar.activation(pT_[:], ps_[:], AF.Exp, scale=0.125)
                        pTs.append(pT_)
                    if prev is not None:
                        emit_pv(prev)
                    prev = (pTs, sqc, skc)
            emit_pv(prev)


def build_nc():
    nc = bacc.Bacc("TRN2", target_bir_lowering=False, debug=False,
                   num_devices=NCORES)
    io = {
        "xqT": nc.dram_tensor("xqT", [D, S], BF16, kind="ExternalInput").ap(),
        "xkT": nc.dram_tensor("xkT", [D, S], BF16, kind="ExternalInput").ap(),
        "xvT": nc.dram_tensor("xvT", [D + 1, S], BF16, kind="ExternalInput").ap(),
        "wqT": nc.dram_tensor("wqT", [D, GO], BF16, kind="ExternalInput").ap(),
        "wkT": nc.dram_tensor("wkT", [D, GO], BF16, kind="ExternalInput").ap(),
        "wvT": nc.dram_tensor("wvT", [D + 1, GO], BF16, kind="ExternalInput").ap(),
        "woT": nc.dram_tensor("woT", [GO, D], BF16, kind="ExternalInput").ap(),
        "bqk": nc.dram_tensor("bqk", [128, 4], F32, kind="ExternalInput").ap(),
        "outT": nc.dram_tensor("outT", [D, S], BF16, kind="ExternalOutput").ap(),
    }
    with tile.TileContext(nc) as tc:
        _emit(nc, tc, io)
    nc.compile()
    return nc


def get_nc():
    global _NC
    if _NC is None:
        _NC = build_nc()
    return _NC


def shard_inputs(Q, K, V, Wq, bq, Wk, bk, Wv, bv, Wo, bo):
    bf = ml_dtypes.bfloat16
    ones = np.ones((1, S), np.float32)
    in_maps = []
    for core in range(NCORES):
        b, hg = core // HG, core % HG
        rows = slice(GO * hg, GO * hg + GO)
        bq_g, bk_g, bv_g = bq[rows], bk[rows], bv[rows]
        bqk_t = np.stack(
            [bq_g[0:128], bq_g[128:256], bk_g[0:128], bk_g[128:256]], axis=1
        ).astype(np.float32)
        in_maps.append({
            "xqT": np.ascontiguousarray(Q[b].T).astype(bf),
            "xkT": np.ascontiguousarray(K[b].T).astype(bf),
            "xvT": np.concatenate([V[b].T, ones], 0).astype(bf),
            "wqT": np.ascontiguousarray(Wq[rows].T).astype(bf),
            "wkT": np.ascontiguousarray(Wk[rows].T).astype(bf),
            "wvT": np.concatenate([Wv[rows].T, bv_g[None, :]], 0).astype(bf),
            "woT": np.ascontiguousarray(Wo[:, rows].T).astype(bf),
            "bqk": bqk_t,
        })
    return in_maps


def kernel(**inputs):
    args = {k: np.asarray(v) for k, v in inputs.items()}
    nc = get_nc()
    in_maps = shard_inputs(
        args["Q"], args["K"], args["V"], args["Wq"], args["bq"], args["Wk"],
        args["bk"], args["Wv"], args["bv"], args["Wo"], args["bo"],
    )
    res = run_bass_kernel_spmd(nc, in_maps, list(range(NCORES)))
    out = np.zeros((B, S, D), np.float32)
    for core in range(NCORES):
        out[core // HG] += res.results[core]["outT"].astype(np.float32).T
    out += args["bo"].astype(np.float32)
    return out


# revision 12
# speedup vs baseline: 1.1132x; 1.1132x over previous
"""Multi-head attention (B=2, S=2048, D=1024, H=16) on 8 Trainium2 NeuronCores.

Sharding: core = b*4 + hg  (b = batch, hg = head-group of 4 heads).
Each core computes, for its batch b and its 4 heads:
    q^T = (Wq_g @ X_q^T + bq_g)      stored [256, 2048]  (head-dim on partitions)
    k^T likewise, v = X_v @ Wv_g^T + bv_g stored [2048, 4, 64]
    S^T[s_k, s_q] = k^T.T-contraction(d)  per head (row-packed PE pairs)
    P^T = exp(S^T / 8)                (scalar engine, exp only; no max-subtract)
    attnout^T[o, s_q] = v.T @ P^T     col-packed PE pairs (M=64 per head, two
                                      heads share one PE pass)
    rowsum[s_q]      = 1.T @ P^T      4-way col-packed M=1 matmuls into one
                                      PSUM bank (partitions 0/32/64/96)
    attnout_norm^T = attnout^T * (1/rowsum)  (DVE; reciprocal via DRAM-bounce
                                      transpose so it runs 128-lane-parallel)
    outT_partial[m, s] = Wo_g^T-contraction(o) @ attnout_norm^T   [1024, 2048]
Host gathers: out[b] = sum_g outT_partial.T + bo.

Engine split: scalar = exp only (it is the 1 elem/cycle/lane bottleneck);
DVE = all PSUM drains + normalize; gpsimd/sync = DMA queues.
All matmul inputs bf16 (PSUM accumulation f32).
"""

import numpy as np
import ml_dtypes

import concourse.bacc as bacc
import concourse.mybir as mybir
import concourse.tile as tile
from concourse.bass_utils import run_bass_kernel_spmd

BF16 = mybir.dt.bfloat16
F32 = mybir.dt.float32
AF = mybir.ActivationFunctionType
ALU = mybir.AluOpType

B, S, D = 2, 2048, 1024
H = 16
DK = 64
NCORES = 8
HG = 4  # head groups
HPG = 4  # heads per group
GO = HPG * DK  # 256 group output width

_NC = None


def _emit(nc, tc, io):
    xqT, xkT, xvT, wqT, wkT, wvT, woT, bqk, outT = (
        io["xqT"], io["xkT"], io["xvT"], io["wqT"], io["wkT"], io["wvT"],
        io["woT"], io["bqk"], io["outT"],
    )
    NIC = D // 128  # 8 contraction chunks of 128
    NSC = S // 128  # 16 s chunks of 128
    NSQ = S // 512  # 4 s chunks of 512

    with (
        tc.tile_pool(name="wp", bufs=1) as wp,
        tc.tile_pool(name="xp", bufs=1) as xp,
        tc.tile_pool(name="pp", bufs=1) as pp,
        tc.tile_pool(name="pt", bufs=12) as ptp,
        tc.tile_pool(name="nr", bufs=2) as nrp,
        tc.tile_pool(name="sc", bufs=2, space="PSUM") as psB,
        tc.tile_pool(name="dr", bufs=2, space="DRAM") as drp,
    ):
        # ---- load weights + inputs (in consumption order) ----
        bqk_t = wp.tile([128, 4], F32, name="bqk", tag="bqk")
        nc.sync.dma_start(bqk_t[:], bqk[:])
        ones_t = wp.tile([128, 1], BF16, name="ones", tag="ones")
        nc.vector.memset(ones_t[:], 1.0)
        wk = []
        xk = []
        for i in range(NIC):
            t = wp.tile([128, GO], BF16, name=f"wk{i}", tag=f"wk{i}")
            nc.sync.dma_start(t[:], wkT[128 * i:128 * i + 128, :])
            wk.append(t)
            t = xp.tile([128, S], BF16, name=f"xk{i}", tag=f"xk{i}")
            nc.gpsimd.dma_start(t[:], xkT[128 * i:128 * i + 128, :])
            xk.append(t)
        wq = []
        xq = []
        for i in range(NIC):
            t = wp.tile([128, GO], BF16, name=f"wq{i}", tag=f"wq{i}")
            nc.sync.dma_start(t[:], wqT[128 * i:128 * i + 128, :])
            wq.append(t)
            t = xp.tile([128, S], BF16, name=f"xq{i}", tag=f"xq{i}")
            nc.gpsimd.dma_start(t[:], xqT[128 * i:128 * i + 128, :])
            xq.append(t)
        wv = []
        xv = []
        for i in range(NIC):
            t = wp.tile([128, GO], BF16, name=f"wv{i}", tag=f"wv{i}")
            nc.sync.dma_start(t[:], wvT[128 * i:128 * i + 128, :])
            wv.append(t)
            t = xp.tile([128, S], BF16, name=f"xv{i}", tag=f"xv{i}")
            nc.gpsimd.dma_start(t[:], xvT[128 * i:128 * i + 128, :])
            xv.append(t)
        wv_b = wp.tile([1, GO], BF16, name="wvb", tag="wvb")
        nc.sync.dma_start(wv_b[:], wvT[D:D + 1, :])
        xv_ones = xp.tile([1, S], BF16, name="xvo", tag="xvo")
        nc.sync.dma_start(xv_ones[:], xvT[D:D + 1, :])
        wo = []
        for oc in range(2):
            t = wp.tile([128, D], BF16, name=f"wo{oc}", tag=f"wo{oc}")
            nc.sync.dma_start(t[:], woT[128 * oc:128 * oc + 128, :])
            wo.append(t)

        kT = [pp.tile([128, S], BF16, name=f"kT{oc}", tag=f"kT{oc}") for oc in range(2)]
        qT = [pp.tile([128, S], BF16, name=f"qT{oc}", tag=f"qT{oc}") for oc in range(2)]
        v = [pp.tile([128, HPG, DK], BF16, name=f"v{sc}", tag=f"v{sc}") for sc in range(NSC)]

        # ---- projections in a scoped PSUM pool (4 banks, released before
        # the attention pools are allocated) ----
        with tc.tile_pool(name="pj", bufs=4, space="PSUM") as pj:
            # k^T: ic-outer with 4 accumulators per oc pass (accumulate as
            # the input DMAs land); bias added in the DVE drain.
            for oc in range(2):
                accs = [pj.tile([128, 512], F32, name="acc", tag="acc")
                        for _ in range(NSQ)]
                for ic in range(NIC):
                    for sc in range(NSQ):
                        nc.tensor.matmul(
                            accs[sc][:],
                            wk[ic][:, 128 * oc:128 * oc + 128],
                            xk[ic][:, 512 * sc:512 * sc + 512],
                            start=(ic == 0),
                            stop=(ic == NIC - 1),
                        )
                for sc in range(NSQ):
                    nc.vector.tensor_scalar(
                        kT[oc][:, 512 * sc:512 * sc + 512], accs[sc][:],
                        bqk_t[:, 2 + oc:3 + oc], None, op0=ALU.add,
                    )
            # q^T: sc-outer 2-acc mini-passes so the sqc=0 slice of qT (the
            # only one the first score blocks need) drains as early as
            # possible and the exp pipeline can start.
            for sc in range(NSQ):
                accs = [pj.tile([128, 512], F32, name="acc", tag="acc")
                        for _ in range(2)]
                for ic in range(NIC):
                    for oc in range(2):
                        nc.tensor.matmul(
                            accs[oc][:],
                            wq[ic][:, 128 * oc:128 * oc + 128],
                            xq[ic][:, 512 * sc:512 * sc + 512],
                            start=(ic == 0),
                            stop=(ic == NIC - 1),
                        )
                for oc in range(2):
                    nc.vector.tensor_scalar(
                        qT[oc][:, 512 * sc:512 * sc + 512], accs[oc][:],
                        bqk_t[:, oc:oc + 1], None, op0=ALU.add,
                    )
            # v: [128, 4, 64] per s-chunk (bias via ones-row matmul)
            for scg in range(NSC // NSQ):
                accs = [pj.tile([128, 512], F32, name="acc", tag="acc")
                        for _ in range(NSQ)]
                for ic in range(NIC):
                    for j in range(NSQ):
                        sc = scg * NSQ + j
                        nc.tensor.matmul(
                            accs[j][:, 0:GO],
                            xv[ic][:, 128 * sc:128 * sc + 128],
                            wv[ic][:],
                            start=(ic == 0),
                            stop=False,
                        )
                for j in range(NSQ):
                    sc = scg * NSQ + j
                    nc.tensor.matmul(
                        accs[j][:, 0:GO],
                        xv_ones[:, 128 * sc:128 * sc + 128],
                        wv_b[:],
                        start=False,
                        stop=True,
                    )
                    nc.vector.tensor_copy(
                        v[sc][:, :, :],
                        accs[j][:, 0:GO].rearrange("p (h d) -> p h d", d=DK),
                    )

        # ---- attention ----
        # pvacc pair tiles: partitions 0:64 = even head, 64:128 = odd head
        # (two col-packed matmuls share one PE pass).  rs tile: rowsums at
        # partitions 0/32/64/96 via 4-way col-packed M=1 matmuls.
        # pvacc gets its OWN 2 banks: PV(c+1) must depend only on the atc
        # drains of chunk c, never (via a shared slot ring) on the normalize
        # DMA chain or fproj — otherwise the pT queue fills and the exp
        # pipeline stalls ~20us at every chunk boundary.
        with (
            tc.tile_pool(name="pv", bufs=2, space="PSUM") as pvp,
            tc.tile_pool(name="fx", bufs=1, space="PSUM") as fxp,
            tc.tile_pool(name="rx", bufs=1, space="PSUM") as rxp,
        ):
            attnT = [pp.tile([128, S], BF16, name=f"at{oc}", tag=f"at{oc}")
                     for oc in range(2)]
            pvacc = {}
            rs = {}

            def normalize(c):
                last = c == NSQ - 1
                # full-tile reciprocal straight from the rs PSUM bank: the
                # junk lanes cost nothing and this removes the whole
                # DRAM-transpose bounce (3 DMA hops at 1.5-3us latency each)
                rss_r = nrp.tile([128, 512], F32, name="rssr", tag="rssr", bufs=2)
                nc.vector.reciprocal(rss_r[:], rs[c][:])
                rd_ = drp.tile([4, 512], F32, name="rd", tag="rd")
                for h in range(HPG):
                    nc.sync.dma_start(rd_[h:h + 1, :],
                                      rss_r[32 * h:32 * h + 1, :])
                atcs = []
                if not last:
                    # drain pvacc to SBUF so PV(c+1) can reuse the banks; on
                    # the final chunk multiply straight from PSUM instead
                    for p in range(2):
                        atc = nrp.tile([128, 512], F32, name="atc", tag="atc",
                                       bufs=4)
                        nc.vector.tensor_copy(atc[:], pvacc[c][p][:])
                        atcs.append(atc)
                for p in range(2):
                    rb_ = nrp.tile([128, 512], F32, name="rb", tag="rb", bufs=4)
                    nc.gpsimd.dma_start(
                        rb_[0:DK, :],
                        rd_[2 * p:2 * p + 1, :].to_broadcast([DK, 512]))
                    nc.gpsimd.dma_start(
                        rb_[DK:128, :],
                        rd_[2 * p + 1:2 * p + 2, :].to_broadcast([DK, 512]))
                    st_ = nrp.tile([128, 512], BF16, name="st", tag="st", bufs=4)
                    src = pvacc[c][p][:] if last else atcs[p][:]
                    nc.vector.tensor_mul(st_[:], src, rb_[:])
                    nc.gpsimd.dma_start(
                        attnT[p][:, 512 * c:512 * c + 512],
                        st_[:],
                    )

            def emit_fproj(c):
                last = c == NSQ - 1
                for mc in range(D // 128):
                    if last:
                        # the scores ring is dead by now — borrow its banks
                        # so the tail fproj double-buffers instead of
                        # serializing MM -> drain -> MM
                        fac = psB.tile([128, 1024], F32, name="sc",
                                       tag="sc")[:, 0:512]
                    else:
                        fac = fxp.tile([128, 512], F32, name="fac", tag="fac")
                    for oc in range(2):
                        nc.tensor.matmul(
                            fac,
                            wo[oc][:, 128 * mc:128 * mc + 128],
                            attnT[oc][:, 512 * c:512 * c + 512],
                            start=(oc == 0),
                            stop=(oc == 1),
                        )
                    fo_ = nrp.tile([128, 512], BF16, name="fo", tag="fo", bufs=8)
                    nc.vector.tensor_copy(fo_[:], fac)
                    eng = nc.sync if mc % 2 == 0 else nc.gpsimd
                    eng.dma_start(
                        outT[128 * mc:128 * mc + 128, 512 * c:512 * c + 512],
                        fo_[:],
                    )

            def emit_pv(prev):
                pTs, c, k = prev
                if k == 0:
                    pvacc[c] = [
                        pvp.tile([128, 512], F32, name="pvacc", tag="pv")
                        for _ in range(2)
                    ]
                    rs[c] = rxp.tile([128, 512], F32, name="rs", tag="rs")
                for p in range(2):
                    for sub in range(2):
                        h = 2 * p + sub
                        nc.tensor.matmul(
                            pvacc[c][p][64 * sub:64 * sub + 64, :],
                            v[k][:, h, :],
                            pTs[h // 2][:, 512 * (h % 2):512 * (h % 2) + 512],
                            start=(k == 0),
                            stop=(k == NSC - 1),
                            tile_position=(0, 64 * sub),
                        )
                for h in range(HPG):
                    nc.tensor.matmul(
                        rs[c][32 * h:32 * h + 1, :],
                        ones_t[:, :],
                        pTs[h // 2][:, 512 * (h % 2):512 * (h % 2) + 512],
                        start=(k == 0),
                        stop=(k == NSC - 1),
                        tile_position=(0, 32 * h),
                    )
                if k == NSC - 1:
                    normalize(c)
                    emit_fproj(c)

            # 1-step software pipeline: scores/exp run one step ahead of the
            # PV matmuls; psB double-buffered per head-pair.
            prev = None
            for sqc in range(NSQ):
                for skc in range(NSC):
                    pTs = []
                    for hp in range(2):
                        ps_ = psB.tile([128, 1024], F32, name="sc", tag="sc")
                        for sub in range(2):
                            nc.tensor.matmul(
                                ps_[:, 512 * sub:512 * sub + 512],
                                kT[hp][64 * sub:64 * sub + 64,
                                       128 * skc:128 * skc + 128],
                                qT[hp][64 * sub:64 * sub + 64,
                                       512 * sqc:512 * sqc + 512],
                                start=True,
                                stop=True,
                                tile_position=(64 * sub, 0),
                            )
                        pT_ = ptp.tile([128, 1024], BF16, name="pT", tag="pT")
                        nc.scalar.activation(pT_[:], ps_[:], AF.Exp, scale=0.125)
                        pTs.append(pT_)
                    if prev is not None:
                        emit_pv(prev)
                    prev = (pTs, sqc, skc)
            emit_pv(prev)


def build_nc():
    nc = bacc.Bacc("TRN2", target_bir_lowering=False, debug=False,
                   num_devices=NCORES)
    io = {
        "xqT": nc.dram_tensor("xqT", [D, S], BF16, kind="ExternalInput").ap(),
        "xkT": nc.dram_tensor("xkT", [D, S], BF16, kind="ExternalInput").ap(),
        "xvT": nc.dram_tensor("xvT", [D + 1, S], BF16, kind="ExternalInput").ap(),
        "wqT": nc.dram_tensor("wqT", [D, GO], BF16, kind="ExternalInput").ap(),
        "wkT": nc.dram_tensor("wkT", [D, GO], BF16, kind="ExternalInput").ap(),
        "wvT": nc.dram_tensor("wvT", [D + 1, GO], BF16, kind="ExternalInput").ap(),
        "woT": nc.dram_tensor("woT", [GO, D], BF16, kind="ExternalInput").ap(),
        "bqk": nc.dram_tensor("bqk", [128, 4], F32, kind="ExternalInput").ap(),
        "outT": nc.dram_tensor("outT", [D, S], BF16, kind="ExternalOutput").ap(),
    }
    with tile.TileContext(nc) as tc:
        _emit(nc, tc, io)
    nc.compile()
    return nc


def get_nc():
    global _NC
    if _NC is None:
        _NC = build_nc()
    return _NC


def shard_inputs(Q, K, V, Wq, bq, Wk, bk, Wv, bv, Wo, bo):
    bf = ml_dtypes.bfloat16
    ones = np.ones((1, S), np.float32)
    in_maps = []
    for core in range(NCORES):
        b, hg = core // HG, core % HG
        rows = slice(GO * hg, GO * hg + GO)
        bq_g, bk_g, bv_g = bq[rows], bk[rows], bv[rows]
        bqk_t = np.stack(
            [bq_g[0:128], bq_g[128:256], bk_g[0:128], bk_g[128:256]], axis=1
        ).astype(np.float32)
        in_maps.append({
            "xqT": np.ascontiguousarray(Q[b].T).astype(bf),
            "xkT": np.ascontiguousarray(K[b].T).astype(bf),
            "xvT": np.concatenate([V[b].T, ones], 0).astype(bf),
            "wqT": np.ascontiguousarray(Wq[rows].T).astype(bf),
            "wkT": np.ascontiguousarray(Wk[rows].T).astype(bf),
            "wvT": np.concatenate([Wv[rows].T, bv_g[None, :]], 0).astype(bf),
            "woT": np.ascontiguousarray(Wo[:, rows].T).astype(bf),
            "bqk": bqk_t,
        })
    return in_maps


def kernel(**inputs):
    args = {k: np.asarray(v) for k, v in inputs.items()}
    nc = get_nc()
    in_maps = shard_inputs(
        args["Q"], args["K"], args["V"], args["Wq"], args["bq"], args["Wk"],
        args["bk"], args["Wv"], args["bv"], args["Wo"], args["bo"],
    )
    res = run_bass_kernel_spmd(nc, in_maps, list(range(NCORES)))
    out = np.zeros((B, S, D), np.float32)
    for core in range(NCORES):
        out[core // HG] += res.results[core]["outT"].astype(np.float32).T
    out += args["bo"].astype(np.float32)
    return out
